# revision 49
# baseline (speedup 1.0000x reference)
"""MoE layer (top-2 routing, 16 experts) on 8 Trainium2 NeuronCores.

Strategy: expert-parallel. The gate (logits -> top-2 -> softmax) is computed
on the host as part of the dispatch/sharding step, replicating the reference's
jax ops so routing decisions match bit-for-bit. Tokens are gathered per
expert, experts are paired two-per-core (largest-with-smallest to balance
load), and each core runs the expert FFN (silu(x @ W1e) @ W2e) over its
gathered tokens with weights held resident in SBUF.

Device layout keeps tokens on the matmul free dimension throughout (x is
shipped transposed, [D, tokens]) so no on-chip transposes are needed:
  mm1: A^T[f, tok] += W1[d, f]^T-chunks (stationary) @ x^T[d, tok]
  silu on ScalarE, PSUM -> SBUF
  mm2: y^T[d, tok] += W2[f, d]-chunks (stationary) @ silu(A^T)[f, tok]
Matmuls run in float32r (full PE speed for moving dim >= 256, ~1e-3 rel err).

The host then scatter-gathers the per-(token, slot) columns back and applies
the top-2 combine weights.
"""

import os

import numpy as np

B, T, D, F, E = 4, 2048, 1024, 2048, 16
N_CORES = 8
P = 128
D_TILES = D // P   # 8
F_TILES = F // P   # 16
N_TOK = B * T      # 8192

_nc_cache = {}
last_results = None  # BassKernelResults of the most recent run (for test.py)


def _gate(x, Wg):
    """Top-2 routing. Uses the same jax ops as the reference so the discrete
    expert choice matches it bit-for-bit; falls back to float64 numpy."""
    h = np.asarray(x, dtype=np.float32).reshape(-1, D)
    try:
        import jax
        import jax.numpy as jnp

        logits = jnp.asarray(h) @ jnp.asarray(np.asarray(Wg, dtype=np.float32))
        scores, idx = jax.lax.top_k(logits, 2)
        probs = jax.nn.softmax(scores.astype(jnp.float32), axis=-1)
        return np.asarray(idx), np.asarray(probs, dtype=np.float32)
    except Exception:
        logits = h.astype(np.float64) @ np.asarray(Wg).astype(np.float64)
        idx = np.argsort(-logits, axis=1, kind="stable")[:, :2]
        s = np.take_along_axis(logits, idx, axis=1)
        s = s - s.max(axis=1, keepdims=True)
        p = np.exp(s)
        p /= p.sum(axis=1, keepdims=True)
        return idx.astype(np.int32), p.astype(np.float32)


def _supertiles(count):
    """Decompose a token-count capacity into supertile sizes from
    {512, 384, 256} (>=256 keeps float32r matmuls at full speed) whose sum is
    the smallest feasible multiple of 128 >= count. Largest first."""
    count = max(int(count), 256)
    m = -(-count // 128)
    if m < 2:
        m = 2
    sizes = []
    if m % 2:
        sizes.append(384)
        m -= 3
    while m >= 4:
        sizes.append(512)
        m -= 4
    if m == 2:
        sizes.append(256)
    return tuple(sorted(sizes, reverse=True))


def _build(st0, st1):
    """Build + compile the per-core SPMD program for supertile plans st0/st1."""
    import concourse.bacc as bacc
    import concourse.mybir as mybir
    import concourse.tile as tile

    C0, C1 = sum(st0), sum(st1)
    C = C0 + C1
    SMAX = max(max(st0), max(st1))
    f32r = mybir.dt.float32r
    f32 = mybir.dt.float32

    nc = bacc.Bacc("TRN2", target_bir_lowering=False, debug=False)
    xt = nc.dram_tensor("xt", [D, C], f32r, kind="ExternalInput").ap()
    # weights arrive host-permuted so every chunk DMA is a contiguous copy:
    # w1[e, chunk, p, dt, f'], w2[e, chunk, p, ft, d']
    w1 = nc.dram_tensor("w1", [2, 8, P, D_TILES, F // 8], f32r,
                        kind="ExternalInput").ap()
    w2 = nc.dram_tensor("w2", [2, 8, P, F_TILES, D // 8], f32r,
                        kind="ExternalInput").ap()
    out = nc.dram_tensor("out", [D, C], f32, kind="ExternalOutput").ap()
    scratch = nc.dram_tensor("warm_scratch", [1, 1], f32).ap()

    xt_v = xt.rearrange("(dt p) c -> p dt c", p=P)    # [128, 8, C]
    out_v = out.rearrange("(dt p) c -> p dt c", p=P)

    with tile.TileContext(nc) as tc:
        with (
            tc.tile_pool(name="wpool", bufs=1) as wpool,
            tc.tile_pool(name="xpool", bufs=2) as xpool,
            tc.tile_pool(name="apool", bufs=1) as apool,
            tc.tile_pool(name="opool", bufs=4) as opool,
            tc.tile_pool(name="ps1", bufs=3, space="PSUM") as ps1p,
            tc.tile_pool(name="ps2", bufs=5, space="PSUM") as ps2p,
            tc.tile_pool(name="warm", bufs=1) as warmp,
        ):
            # PE warm-up: the HAM clock gate keeps the PE at 1.2 GHz until it
            # has been busy ~3.4us. Run dummy matmuls on zeroed tiles during
            # the DMA lead-in (no data deps) so real matmuls start at 2.4 GHz.
            wa = warmp.tile([P, P], mybir.dt.bfloat16, tag="wa")
            wb = warmp.tile([P, 512], mybir.dt.bfloat16, tag="wb")
            nc.any.memset(wa[:], 0.0)
            nc.any.memset(wb[:], 0.0)
            pw = ps2p.tile([P, 512], f32, tag="ps2")
            N_WARM = 28
            for k in range(N_WARM):
                nc.tensor.matmul(
                    pw[:], wa[:], wb[:],
                    start=(k == 0), stop=(k == N_WARM - 1),
                )
            # keep the warm-up alive past DCE with a 4-byte scratch store
            wc = warmp.tile([1, 1], f32, tag="wc")
            nc.vector.tensor_copy(wc[:], pw[0:1, 0:1])
            nc.sync.dma_start(scratch[:], wc[:])
            # weight SBUF layout groups chunks contiguously in the free dim so
            # Tile's range tracking sees each matmul depending only on its own
            # chunk's DMA (consumption-ordered streaming).
            W1C, W2C = 8, 8                      # w1: 2 f-tiles/chunk, w2: 1 d-tile/chunk
            F_PER = F // W1C                     # 256
            D_PER = D // W2C                     # 128
            for e, sts, base in ((0, st0, 0), (1, st1, C0)):
                # Queue split: the sync HWDGE queue carries ONLY the big weight
                # streams; tokens/outputs/silu ride the scalar engine so a
                # pending weight load (blocked on the previous expert's last
                # use of the slot) never head-of-line-blocks them.
                # Startup-critical stream on the sync queue, in consumption
                # order: w1 chunk 0, then the first supertile's tokens
                # (per-d-chunk so the first PSUM group starts on chunk 0),
                # then the rest of w1, then w2. Weights use one tile per
                # chunk so the NEXT expert's chunk loads only wait for this
                # expert's last read of that chunk, not of the whole tensor.
                xt_first = xpool.tile([P, D_TILES, SMAX], f32r, tag="xt")
                w1_ch = [
                    wpool.tile([P, D_TILES, F_PER], f32r,
                               tag=f"w1c{i}", name=f"w1c{i}_{e}")
                    for i in range(W1C)
                ]
                w2_ch = [
                    wpool.tile([P, F_TILES, D_PER], f32r,
                               tag=f"w2c{j}", name=f"w2c{j}_{e}")
                    for j in range(W2C)
                ]
                # interleave the first token tile's d-chunks with the w1
                # chunks so neither stream starves the other: mm1's f-groups
                # consume w1 chunks every ~3.4us while the f0 group needs the
                # xt d-chunks up front.
                nc.sync.dma_start(w1_ch[0][:], w1[e, 0])
                w1_next = 1
                for dd in range(D_TILES):
                    nc.sync.dma_start(
                        xt_first[:, dd, :sts[0]],
                        xt_v[:, dd, base:base + sts[0]],
                    )
                    if dd % 2 == 1 and w1_next < W1C:
                        nc.sync.dma_start(w1_ch[w1_next][:], w1[e, w1_next])
                        w1_next += 1
                for i in range(w1_next, W1C):
                    nc.sync.dma_start(w1_ch[i][:], w1[e, i])
                for j in range(W2C):
                    nc.sync.dma_start(w2_ch[j][:], w2[e, j])
                off = base
                xt_next = xt_first
                for si, S in enumerate(sts):
                    xt_t = xt_next
                    at = apool.tile([P, F_TILES, SMAX], f32r, tag="at")
                    for f in range(F_TILES):
                        ps = ps1p.tile([P, SMAX], f32, tag="ps1")
                        for d in range(D_TILES):
                            nc.tensor.matmul(
                                ps[:, :S],
                                w1_ch[f // 2][:, d, (f % 2) * P:(f % 2 + 1) * P],
                                xt_t[:, d, :S],
                                start=(d == 0),
                                stop=(d == D_TILES - 1),
                            )
                        nc.scalar.activation(
                            at[:, f, :S], ps[:, :S],
                            mybir.ActivationFunctionType.Silu,
                        )
                        if f == 7 and si + 1 < len(sts):
                            # prefetch the next supertile's tokens mid-stream:
                            # enqueued here, its buffer-recycle WAR is already
                            # resolved, so it can't head-of-line-block the
                            # output DMAs behind it on this queue.
                            S2 = sts[si + 1]
                            o2 = off + S
                            xt_next = xpool.tile(
                                [P, D_TILES, SMAX], f32r, tag="xt"
                            )
                            # gpsimd SWDGE queue: token prefetches whose WAR
                            # waits are pending must never sit ahead of output
                            # DMAs (scalar queue) or weight streams (sync).
                            nc.gpsimd.dma_start(
                                xt_next[:, :, :S2], xt_v[:, :, o2:o2 + S2]
                            )
                    for d in range(D_TILES):
                        ps = ps2p.tile([P, SMAX], f32, tag="ps2")
                        for f in range(F_TILES):
                            nc.tensor.matmul(
                                ps[:, :S],
                                w2_ch[d][:, f],
                                at[:, f, :S],
                                start=(f == 0),
                                stop=(f == F_TILES - 1),
                            )
                        ot = opool.tile([P, SMAX], f32, tag="ot")
                        nc.vector.tensor_copy(ot[:, :S], ps[:, :S])
                        nc.scalar.dma_start(out_v[:, d, off:off + S], ot[:, :S])
                    off += S
    nc.compile()
    return nc


def kernel(x, Wg, W1, W2):
    global last_results
    import concourse.bass_utils as bass_utils

    x = np.asarray(x, dtype=np.float32)
    W1 = np.asarray(W1, dtype=np.float32)
    W2 = np.asarray(W2, dtype=np.float32)

    idx, probs = _gate(x, Wg)
    h = x.reshape(-1, D)

    counts = np.bincount(idx.ravel(), minlength=E)
    order = np.argsort(-counts, kind="stable")
    pairs = [(int(order[i]), int(order[2 * N_CORES - 1 - i])) for i in range(N_CORES)]
    cap0 = int(counts[[p[0] for p in pairs]].max())
    cap1 = int(counts[[p[1] for p in pairs]].max())
    st0, st1 = _supertiles(cap0), _supertiles(cap1)
    C0, C1 = sum(st0), sum(st1)
    C = C0 + C1

    key = (st0, st1)
    nc = _nc_cache.get(key)
    if nc is None:
        nc = _build(st0, st1)
        _nc_cache[key] = nc

    pos = np.empty((N_TOK, 2), np.int64)
    in_maps = []
    for c, (e0, e1) in enumerate(pairs):
        ids = np.zeros(C, np.int64)
        for e, off in ((e0, 0), (e1, C0)):
            tok = np.nonzero((idx[:, 0] == e) | (idx[:, 1] == e))[0]
            ids[off:off + len(tok)] = tok
            first = idx[tok, 0] == e
            gcol = c * C + off + np.arange(len(tok))
            pos[tok[first], 0] = gcol[first]
            pos[tok[~first], 1] = gcol[~first]
        # permute weights to [e, chunk, p, tiles, cols] so each chunk DMA is
        # one contiguous copy (see _build)
        w1c = (
            W1[[e0, e1]]
            .reshape(2, D_TILES, P, 8, F // 8)
            .transpose(0, 3, 2, 1, 4)
        )
        w2c = (
            W2[[e0, e1]]
            .reshape(2, F_TILES, P, 8, D // 8)
            .transpose(0, 3, 2, 1, 4)
        )
        in_maps.append({
            "xt": np.ascontiguousarray(h[ids].T),
            "w1": np.ascontiguousarray(w1c),
            "w2": np.ascontiguousarray(w2c),
        })

    trace = os.environ.get("MOE_TRACE") == "1"
    kwargs = {}
    if trace:
        kwargs = {"trace": True, "trace_cores": list(range(N_CORES))}
    res = bass_utils.run_bass_kernel_spmd(
        nc, in_maps, core_ids=list(range(N_CORES)), **kwargs
    )
    last_results = res

    out_all = np.concatenate([r["out"] for r in res.results], axis=1)  # [D, 8*C]
    y = out_all[:, pos[:, 0]] * probs[:, 0] + out_all[:, pos[:, 1]] * probs[:, 1]
    return np.ascontiguousarray(y.T).reshape(B, T, D).astype(np.float32)


# revision 51
# speedup vs baseline: 1.0029x; 1.0029x over previous
"""MoE layer (top-2 routing, 16 experts) on 8 Trainium2 NeuronCores.

Strategy: expert-parallel. The gate (logits -> top-2 -> softmax) is computed
on the host as part of the dispatch/sharding step, replicating the reference's
jax ops so routing decisions match bit-for-bit. Tokens are gathered per
expert, experts are paired two-per-core (largest-with-smallest to balance
load), and each core runs the expert FFN (silu(x @ W1e) @ W2e) over its
gathered tokens with weights held resident in SBUF.

Device layout keeps tokens on the matmul free dimension throughout (x is
shipped transposed, [D, tokens]) so no on-chip transposes are needed:
  mm1: A^T[f, tok] += W1[d, f]^T-chunks (stationary) @ x^T[d, tok]
  silu on ScalarE, PSUM -> SBUF
  mm2: y^T[d, tok] += W2[f, d]-chunks (stationary) @ silu(A^T)[f, tok]
Matmuls run in float32r (full PE speed for moving dim >= 256, ~1e-3 rel err).

The host then scatter-gathers the per-(token, slot) columns back and applies
the top-2 combine weights.
"""

import os

import numpy as np

B, T, D, F, E = 4, 2048, 1024, 2048, 16
N_CORES = 8
P = 128
D_TILES = D // P   # 8
F_TILES = F // P   # 16
N_TOK = B * T      # 8192

_nc_cache = {}
last_results = None  # BassKernelResults of the most recent run (for test.py)


def _gate(x, Wg):
    """Top-2 routing. Uses the same jax ops as the reference so the discrete
    expert choice matches it bit-for-bit; falls back to float64 numpy."""
    h = np.asarray(x, dtype=np.float32).reshape(-1, D)
    try:
        import jax
        import jax.numpy as jnp

        logits = jnp.asarray(h) @ jnp.asarray(np.asarray(Wg, dtype=np.float32))
        scores, idx = jax.lax.top_k(logits, 2)
        probs = jax.nn.softmax(scores.astype(jnp.float32), axis=-1)
        return np.asarray(idx), np.asarray(probs, dtype=np.float32)
    except Exception:
        logits = h.astype(np.float64) @ np.asarray(Wg).astype(np.float64)
        idx = np.argsort(-logits, axis=1, kind="stable")[:, :2]
        s = np.take_along_axis(logits, idx, axis=1)
        s = s - s.max(axis=1, keepdims=True)
        p = np.exp(s)
        p /= p.sum(axis=1, keepdims=True)
        return idx.astype(np.int32), p.astype(np.float32)


def _supertiles(count):
    """Decompose a token-count capacity into supertile sizes from
    {512, 384, 256} (>=256 keeps float32r matmuls at full speed) whose sum is
    the smallest feasible multiple of 128 >= count. Largest first."""
    count = max(int(count), 256)
    m = -(-count // 128)
    if m < 2:
        m = 2
    sizes = []
    if m % 2:
        sizes.append(384)
        m -= 3
    while m >= 4:
        sizes.append(512)
        m -= 4
    if m == 2:
        sizes.append(256)
    return tuple(sorted(sizes, reverse=True))


def _build(st0, st1):
    """Build + compile the per-core SPMD program for supertile plans st0/st1."""
    import concourse.bacc as bacc
    import concourse.mybir as mybir
    import concourse.tile as tile

    C0, C1 = sum(st0), sum(st1)
    C = C0 + C1
    SMAX = max(max(st0), max(st1))
    f32r = mybir.dt.float32r
    f32 = mybir.dt.float32

    nc = bacc.Bacc("TRN2", target_bir_lowering=False, debug=False)
    xt = nc.dram_tensor("xt", [D, C], f32r, kind="ExternalInput").ap()
    # weights arrive host-permuted so every chunk DMA is a contiguous copy:
    # w1[e, chunk, p, dt, f'], w2[e, chunk, p, ft, d']
    w1 = nc.dram_tensor("w1", [2, 8, P, D_TILES, F // 8], f32r,
                        kind="ExternalInput").ap()
    w2 = nc.dram_tensor("w2", [2, 8, P, F_TILES, D // 8], f32r,
                        kind="ExternalInput").ap()
    out = nc.dram_tensor("out", [D, C], f32, kind="ExternalOutput").ap()
    scratch = nc.dram_tensor("warm_scratch", [1, 1], f32).ap()

    xt_v = xt.rearrange("(dt p) c -> p dt c", p=P)    # [128, 8, C]
    out_v = out.rearrange("(dt p) c -> p dt c", p=P)

    with tile.TileContext(nc) as tc:
        with (
            tc.tile_pool(name="wpool", bufs=1) as wpool,
            tc.tile_pool(name="xpool", bufs=2) as xpool,
            tc.tile_pool(name="apool", bufs=1) as apool,
            tc.tile_pool(name="opool", bufs=4) as opool,
            tc.tile_pool(name="ps1", bufs=3, space="PSUM") as ps1p,
            tc.tile_pool(name="ps2", bufs=5, space="PSUM") as ps2p,
            tc.tile_pool(name="warm", bufs=1) as warmp,
        ):
            # PE warm-up: the HAM clock gate keeps the PE at 1.2 GHz until it
            # has been busy ~3.4us. Run dummy matmuls on zeroed tiles during
            # the DMA lead-in (no data deps) so real matmuls start at 2.4 GHz.
            wa = warmp.tile([P, P], mybir.dt.bfloat16, tag="wa")
            wb = warmp.tile([P, 512], mybir.dt.bfloat16, tag="wb")
            nc.any.memset(wa[:], 0.0)
            nc.any.memset(wb[:], 0.0)
            pw = ps2p.tile([P, 512], f32, tag="ps2")
            N_WARM = 40
            for k in range(N_WARM):
                nc.tensor.matmul(
                    pw[:], wa[:], wb[:],
                    start=(k == 0), stop=(k == N_WARM - 1),
                )
            # keep the warm-up alive past DCE with a 4-byte scratch store
            wc = warmp.tile([1, 1], f32, tag="wc")
            nc.vector.tensor_copy(wc[:], pw[0:1, 0:1])
            nc.sync.dma_start(scratch[:], wc[:])
            # weight SBUF layout groups chunks contiguously in the free dim so
            # Tile's range tracking sees each matmul depending only on its own
            # chunk's DMA (consumption-ordered streaming).
            W1C, W2C = 8, 8                      # w1: 2 f-tiles/chunk, w2: 1 d-tile/chunk
            F_PER = F // W1C                     # 256
            D_PER = D // W2C                     # 128
            for e, sts, base in ((0, st0, 0), (1, st1, C0)):
                # Queue split: the sync HWDGE queue carries ONLY the big weight
                # streams; tokens/outputs/silu ride the scalar engine so a
                # pending weight load (blocked on the previous expert's last
                # use of the slot) never head-of-line-blocks them.
                # Startup-critical stream on the sync queue, in consumption
                # order: w1 chunk 0, then the first supertile's tokens
                # (per-d-chunk so the first PSUM group starts on chunk 0),
                # then the rest of w1, then w2. Weights use one tile per
                # chunk so the NEXT expert's chunk loads only wait for this
                # expert's last read of that chunk, not of the whole tensor.
                xt_first = xpool.tile([P, D_TILES, SMAX], f32r, tag="xt")
                w1_ch = [
                    wpool.tile([P, D_TILES, F_PER], f32r,
                               tag=f"w1c{i}", name=f"w1c{i}_{e}")
                    for i in range(W1C)
                ]
                w2_ch = [
                    wpool.tile([P, F_TILES, D_PER], f32r,
                               tag=f"w2c{j}", name=f"w2c{j}_{e}")
                    for j in range(W2C)
                ]
                nc.sync.dma_start(w1_ch[0][:], w1[e, 0])
                for dd in range(D_TILES):
                    nc.sync.dma_start(
                        xt_first[:, dd, :sts[0]],
                        xt_v[:, dd, base:base + sts[0]],
                    )
                for i in range(1, W1C):
                    nc.sync.dma_start(w1_ch[i][:], w1[e, i])
                for j in range(W2C):
                    nc.sync.dma_start(w2_ch[j][:], w2[e, j])
                off = base
                xt_next = xt_first
                for si, S in enumerate(sts):
                    xt_t = xt_next
                    at = apool.tile([P, F_TILES, SMAX], f32r, tag="at")
                    for f in range(F_TILES):
                        ps = ps1p.tile([P, SMAX], f32, tag="ps1")
                        for d in range(D_TILES):
                            nc.tensor.matmul(
                                ps[:, :S],
                                w1_ch[f // 2][:, d, (f % 2) * P:(f % 2 + 1) * P],
                                xt_t[:, d, :S],
                                start=(d == 0),
                                stop=(d == D_TILES - 1),
                            )
                        nc.scalar.activation(
                            at[:, f, :S], ps[:, :S],
                            mybir.ActivationFunctionType.Silu,
                        )
                        if f == 7 and si + 1 < len(sts):
                            # prefetch the next supertile's tokens mid-stream:
                            # enqueued here, its buffer-recycle WAR is already
                            # resolved, so it can't head-of-line-block the
                            # output DMAs behind it on this queue.
                            S2 = sts[si + 1]
                            o2 = off + S
                            xt_next = xpool.tile(
                                [P, D_TILES, SMAX], f32r, tag="xt"
                            )
                            # gpsimd SWDGE queue: token prefetches whose WAR
                            # waits are pending must never sit ahead of output
                            # DMAs (scalar queue) or weight streams (sync).
                            nc.gpsimd.dma_start(
                                xt_next[:, :, :S2], xt_v[:, :, o2:o2 + S2]
                            )
                    for d in range(D_TILES):
                        ps = ps2p.tile([P, SMAX], f32, tag="ps2")
                        for f in range(F_TILES):
                            nc.tensor.matmul(
                                ps[:, :S],
                                w2_ch[d][:, f],
                                at[:, f, :S],
                                start=(f == 0),
                                stop=(f == F_TILES - 1),
                            )
                        ot = opool.tile([P, SMAX], f32, tag="ot")
                        nc.vector.tensor_copy(ot[:, :S], ps[:, :S])
                        nc.scalar.dma_start(out_v[:, d, off:off + S], ot[:, :S])
                    off += S
    nc.compile()
    return nc


def kernel(x, Wg, W1, W2):
    global last_results
    import concourse.bass_utils as bass_utils

    x = np.asarray(x, dtype=np.float32)
    W1 = np.asarray(W1, dtype=np.float32)
    W2 = np.asarray(W2, dtype=np.float32)

    idx, probs = _gate(x, Wg)
    h = x.reshape(-1, D)

    counts = np.bincount(idx.ravel(), minlength=E)
    order = np.argsort(-counts, kind="stable")
    pairs = [(int(order[i]), int(order[2 * N_CORES - 1 - i])) for i in range(N_CORES)]
    cap0 = int(counts[[p[0] for p in pairs]].max())
    cap1 = int(counts[[p[1] for p in pairs]].max())
    st0, st1 = _supertiles(cap0), _supertiles(cap1)
    C0, C1 = sum(st0), sum(st1)
    C = C0 + C1

    key = (st0, st1)
    nc = _nc_cache.get(key)
    if nc is None:
        nc = _build(st0, st1)
        _nc_cache[key] = nc

    pos = np.empty((N_TOK, 2), np.int64)
    in_maps = []
    for c, (e0, e1) in enumerate(pairs):
        ids = np.zeros(C, np.int64)
        for e, off in ((e0, 0), (e1, C0)):
            tok = np.nonzero((idx[:, 0] == e) | (idx[:, 1] == e))[0]
            ids[off:off + len(tok)] = tok
            first = idx[tok, 0] == e
            gcol = c * C + off + np.arange(len(tok))
            pos[tok[first], 0] = gcol[first]
            pos[tok[~first], 1] = gcol[~first]
        # permute weights to [e, chunk, p, tiles, cols] so each chunk DMA is
        # one contiguous copy (see _build)
        w1c = (
            W1[[e0, e1]]
            .reshape(2, D_TILES, P, 8, F // 8)
            .transpose(0, 3, 2, 1, 4)
        )
        w2c = (
            W2[[e0, e1]]
            .reshape(2, F_TILES, P, 8, D // 8)
            .transpose(0, 3, 2, 1, 4)
        )
        in_maps.append({
            "xt": np.ascontiguousarray(h[ids].T),
            "w1": np.ascontiguousarray(w1c),
            "w2": np.ascontiguousarray(w2c),
        })

    trace = os.environ.get("MOE_TRACE") == "1"
    kwargs = {}
    if trace:
        kwargs = {"trace": True, "trace_cores": list(range(N_CORES))}
    res = bass_utils.run_bass_kernel_spmd(
        nc, in_maps, core_ids=list(range(N_CORES)), **kwargs
    )
    last_results = res

    out_all = np.concatenate([r["out"] for r in res.results], axis=1)  # [D, 8*C]
    y = out_all[:, pos[:, 0]] * probs[:, 0] + out_all[:, pos[:, 1]] * probs[:, 1]
    return np.ascontiguousarray(y.T).reshape(B, T, D).astype(np.float32)
